# revision 30
# baseline (speedup 1.0000x reference)
"""Self-contained Trainium2 kernel for nn_MultiHeadAttention_91070486544496.

B=4, S=2048, D=1024, H=16 causal MHA. 8-core SPMD: head-parallel
QKV+attention (2 heads/core), mid-attention AllToAll reshard, then
position-parallel output projection. See build() docstring for details.
"""
import sys

for _p in ("/opt/trn_rl_repo", "/root/.axon_site/_ro/trn_rl_repo"):
    if _p not in sys.path:
        sys.path.append(_p)

import numpy as np

# ======== runtime infra (axon NTFF hook, BIR wait splitter) ========

import contextlib
import ctypes
import json
import sys
import types

_SO_PATH = "/opt/axon/libaxon_pjrt.so"


def _ntff_profile_via_ctypes(so_path):
    lib = ctypes.CDLL(so_path)
    if not hasattr(lib, "axon_start_nrt_profile"):
        return None
    lib.axon_start_nrt_profile.argtypes = [
        ctypes.POINTER(ctypes.c_int64),
        ctypes.c_size_t,
    ]
    lib.axon_start_nrt_profile.restype = ctypes.c_int64
    lib.axon_stop_nrt_profile.argtypes = [ctypes.c_char_p]
    lib.axon_stop_nrt_profile.restype = ctypes.c_int64

    @contextlib.contextmanager
    def _hook(output_dir, device_ids):
        import jax
        jax.devices()
        if device_ids:
            ids = (ctypes.c_int64 * len(device_ids))(*device_ids)
            rc = lib.axon_start_nrt_profile(ids, len(device_ids))
        else:
            rc = lib.axon_start_nrt_profile(None, 0)
        if rc != 0:
            raise RuntimeError(f"axon_start_nrt_profile rc={rc}")
        try:
            yield
        finally:
            n = lib.axon_stop_nrt_profile(str(output_dir).encode())
            if n < 0:
                raise RuntimeError(f"axon_stop_nrt_profile rc={n}")

    return _hook


def split_multi_waits(bir_json: bytes) -> bytes:
    d = json.loads(bir_json)
    n_split = 0
    for fn in d.get("functions", []):
        for blk in fn.get("blocks", []):
            insts = blk.get("instructions", [])
            out = []
            for inst in insts:
                si = inst.get("sync_info")
                waits = (si or {}).get("on_wait") or []
                if len(waits) > 1:
                    extra, keep = waits[:-1], waits[-1:]
                    for k, w in enumerate(extra):
                        out.append({
                            "debug": inst.get("debug", 0),
                            "engine": inst["engine"],
                            "ins": [],
                            "outs": [],
                            "name": f"{inst['name']}-ws{k}",
                            "opcode": "NoOp",
                            "sync_info": {"on_update": [], "on_wait": [w]},
                        })
                        n_split += 1
                    si["on_wait"] = keep
                out.append(inst)
            blk["instructions"] = out
    if n_split:
        print(f"bass_infra: split {n_split} extra sync waits into NoOps")
    return json.dumps(d).encode()


def install():
    # 1. antenv.axon_hooks shim
    if "antenv.axon_hooks" not in sys.modules:
        mod = types.ModuleType("antenv.axon_hooks")
        _state = {"hook": _ntff_profile_via_ctypes(_SO_PATH)}
        mod.set_axon_ntff_profile_hook = lambda h: _state.__setitem__("hook", h)
        mod.get_axon_ntff_profile_hook = lambda: _state["hook"]
        sys.modules["antenv.axon_hooks"] = mod
        import antenv
        antenv.axon_hooks = mod

    from concourse import bass_utils, bass2jax

    # 2. upload_artifacts stub
    bass_utils.upload_artifacts = lambda tmpdir: tmpdir

    # 3. wait-splitting compile wrapper
    orig_compile = bass_utils.compile_bir_kernel

    def compile_with_split(bir_json, tmpdir, neff_name="file.neff"):
        return orig_compile(split_multi_waits(bir_json), tmpdir, neff_name=neff_name)

    if getattr(bass2jax.compile_bir_kernel, "__name__", "") != "compile_with_split":
        bass_utils.compile_bir_kernel = compile_with_split
        bass2jax.compile_bir_kernel = compile_with_split


# ======== kernel IR builder ========
from contextlib import ExitStack

import concourse.bass as bass
import concourse.mybir as mybir
import concourse.tile as tile
from concourse.bass import ds, ts
from concourse.masks import make_identity
F32 = mybir.dt.float32
F32R = mybir.dt.float32r
BF16 = mybir.dt.bfloat16
EXP = mybir.ActivationFunctionType.Exp
LN = mybir.ActivationFunctionType.Ln

B, S, D, H, DK = 4, 2048, 1024, 16, 64
NC = 8          # cores
HL = 2          # heads per core
BS = B * S      # 8192
NQ = S // 512   # q-chunks per batch = 4
NKC = S // 128  # k-chunks per batch = 16
NDC = D // 128  # d_in chunks = 8
POS = BS // NC  # positions per core for out-proj = 1024
STR = 128       # stripe width per (batch, half)
NEG = -2.0**33  # pre-exp mask fill (exact in bf16); *0.125 -> -1e9 -> exp == 0


def build(cfg=None):
    cfg = cfg or {}
    nc = bass.Bass("TRN2", target_bir_lowering=False, debug=False, num_devices=NC)

    xT = nc.dram_tensor("xT", [D, BS], BF16, kind="ExternalInput")
    wq = nc.dram_tensor("wq", [D, 128], BF16, kind="ExternalInput")
    wk = nc.dram_tensor("wk", [D, 128], BF16, kind="ExternalInput")
    wv = nc.dram_tensor("wv", [D, 128], BF16, kind="ExternalInput")
    wo = nc.dram_tensor("wo", [D, D], BF16, kind="ExternalInput")
    out = nc.dram_tensor("out", [POS, D], F32, kind="ExternalOutput")

    a2a_in = [nc.dram_tensor(f"a2a_in{t}", [NC, 128, STR], BF16) for t in range(2 * B)]
    a2a_out = [nc.dram_tensor(f"a2a_out{t}", [NC, 128, STR], BF16) for t in range(2 * B)]
    # final A2A (b=3,h=1) split into two partition-half collectives so the
    # first half overlaps the last head's attention compute
    a2a_in7h = [nc.dram_tensor(f"a2a_in7h{i}", [NC, 64, STR], BF16) for i in range(2)]
    a2a_out7h = [nc.dram_tensor(f"a2a_out7h{i}", [NC, 64, STR], BF16) for i in range(2)]

    with tile.TileContext(nc) as tc, ExitStack() as ctx:
        const = ctx.enter_context(tc.tile_pool(name="const", bufs=1))
        wpool = ctx.enter_context(tc.tile_pool(name="wpool", bufs=1))
        xpool = ctx.enter_context(tc.tile_pool(name="xpool", bufs=3))
        qkv_ps = ctx.enter_context(tc.tile_pool(name="qkv_ps", bufs=2, space="PSUM"))
        qk_sb = ctx.enter_context(tc.tile_pool(name="qk_sb", bufs=2))
        vpool = ctx.enter_context(tc.tile_pool(name="vpool", bufs=2))
        sp_ps = ctx.enter_context(tc.tile_pool(name="sp_ps", bufs=2, space="PSUM"))
        et_sb = ctx.enter_context(tc.tile_pool(name="et_sb", bufs=5))
        cp_ps = ctx.enter_context(tc.tile_pool(name="cp_ps", bufs=2, space="PSUM"))
        ep_sb = ctx.enter_context(tc.tile_pool(name="ep_sb", bufs=2))
        epi_sb = ctx.enter_context(tc.tile_pool(name="epi_sb", bufs=1))
        ctx_sb = ctx.enter_context(tc.tile_pool(name="ctx_sb", bufs=2))

        # ---- constants ----
        identf = const.tile([128, 128], F32)
        make_identity(nc, identf[:])
        ident = const.tile([128, 128], BF16)
        nc.vector.tensor_copy(ident[:], identf[:])
        onesf = const.tile([128, 16], BF16)
        nc.vector.memset(onesf[:], 1.0)
        ones_l = const.tile([1, 64], F32)
        nc.vector.memset(ones_l[:], 1.0)
        ones_lr = const.tile([1, 64], BF16)
        nc.vector.tensor_copy(ones_lr[:], ones_l[:])

        # ---- weights ----
        wq_sb = wpool.tile([128, NDC, 128], BF16)
        wk_sb = wpool.tile([128, NDC, 128], BF16)
        wv_sb = wpool.tile([128, NDC, 128], BF16)
        nc.sync.dma_start(wq_sb[:], wq.rearrange("(j p) h -> p j h", p=128))
        nc.sync.dma_start(wk_sb[:], wk.rearrange("(j p) h -> p j h", p=128))
        nc.sync.dma_start(wv_sb[:], wv.rearrange("(j p) h -> p j h", p=128))
        wo_sb = wpool.tile([128, NDC, D], BF16)

        def trigger_a2a(t, ctxT, h):
            nc.sync.dma_start(
                a2a_in[t].rearrange("j p s -> p j s"),
                ctxT[:, ds(h * 1024, NC * STR)].rearrange(
                    "p (j s) -> p j s", j=NC),
            )
            nc.gpsimd.collective_compute(
                "AllToAll", mybir.AluOpType.bypass,
                replica_groups=[list(range(NC))],
                ins=[a2a_in[t][:]], outs=[a2a_out[t][:]],
            )

        def trigger_a2a_half(i, ctxT, h):
            nc.sync.dma_start(
                a2a_in7h[i].rearrange("j p s -> p j s"),
                ctxT[ds(64 * i, 64), ds(h * 1024, NC * STR)].rearrange(
                    "p (j s) -> p j s", j=NC),
            )
            nc.gpsimd.collective_compute(
                "AllToAll", mybir.AluOpType.bypass,
                replica_groups=[list(range(NC))],
                ins=[a2a_in7h[i][:]], outs=[a2a_out7h[i][:]],
            )

        def consume_a2a(t):
            ctxg = ctx_sb.tile([128, NC, STR], BF16, tag="ctxg")
            if t == 2 * B - 1:
                for i in range(2):
                    nc.sync.dma_start(
                        ctxg[ds(64 * i, 64), :, :],
                        a2a_out7h[i].rearrange("j p s -> p j s"))
            else:
                nc.sync.dma_start(
                    ctxg[:], a2a_out[t].rearrange("j p s -> p j s"))
            for nn in range(2):
                op = qkv_ps.tile([128, 512], F32, tag="qkv")
                for j in range(NC):
                    nc.tensor.matmul(
                        op[:], ctxg[:, j, :], wo_sb[:, j, ts(nn, 512)],
                        start=(j == 0), stop=(j == NC - 1),
                    )
                os_ = ep_sb.tile([128, 512], F32, tag="os")
                nc.vector.tensor_copy(os_[:], op[:])
                nc.sync.dma_start(out[ds(t * STR, STR), ts(nn, 512)], os_[:])

        def qkv_batch(b, qt, kt, vaug, first=False):
            nc.vector.tensor_copy(vaug[:, :, 64:65].opt(), onesf[:, 0:NKC])
            nc.vector.tensor_copy(vaug[:, :, 129:130].opt(), onesf[:, 0:NKC])
            for i in range(NQ):  # 512-position chunks
                xt = xpool.tile([128, NDC, 512], BF16)
                col0 = b * S + i * 512
                xsrc = xT.rearrange("(j p) n -> p j n", p=128)[:, :, ds(col0, 512)]
                if first and i == 0:
                    # per-j sub-DMAs so the first matmul starts early
                    for j in range(NDC):
                        nc.sync.dma_start(xt[:, j, :], xsrc[:, j, :])
                else:
                    nc.sync.dma_start(xt[:], xsrc)
                qp = qkv_ps.tile([128, 512], F32, tag="qkv")
                kp = qkv_ps.tile([128, 512], F32, tag="qkv")
                vp = qkv_ps.tile([128, 512], F32, tag="qkv")
                for j in range(NDC):
                    nc.tensor.matmul(qp[:], wq_sb[:, j, :], xt[:, j, :],
                                     start=(j == 0), stop=(j == NDC - 1))
                for j in range(NDC):
                    nc.tensor.matmul(kp[:], wk_sb[:, j, :], xt[:, j, :],
                                     start=(j == 0), stop=(j == NDC - 1))
                for j in range(NDC):
                    nc.tensor.matmul(vp[:], wv_sb[:, j, :], xt[:, j, :],
                                     start=(j == 0), stop=(j == NDC - 1))
                nc.vector.tensor_copy(qt[:, ts(i, 512)], qp[:])
                nc.vector.tensor_copy(kt[:, ts(i, 512)], kp[:])
                # V: transpose [128,128] blocks into [pos, dk] layout
                vs = ep_sb.tile([128, 512], BF16, tag="vs")
                nc.vector.tensor_copy(vs[:], vp[:])
                for j4 in range(4):
                    ki = i * 4 + j4
                    vtp = qkv_ps.tile([128, 512], BF16, tag="qkv")
                    nc.tensor.transpose(vtp[:, 0:128], vs[:, ts(j4, 128)], ident[:])
                    nc.vector.tensor_copy(vaug[:, ki, 0:64], vtp[:, 0:64])
                    nc.vector.tensor_copy(vaug[:, ki, 65:129], vtp[:, 64:128])

        def attn_batch(b, qt, kt, vaug, ctxT, mid_cb, end_cb, half_cb=None):
            def epilogue(cp, hh, qi):
                # softmax normalization: 1/denom = exp(-ln(denom)) on ACT
                lg = ep_sb.tile([1, 512], F32, tag="lg")
                nc.scalar.activation(lg[:], cp[64:65, :], LN)
                rr = ep_sb.tile([1, 512], BF16, tag="rr")
                nc.scalar.activation(rr[:], lg[:], EXP, scale=-1.0)
                bcp = sp_ps.tile([128, 1024], F32, tag="sp")
                nc.tensor.matmul(bcp[0:64, 0:512], ones_lr[:], rr[:],
                                 start=True, stop=True)
                bcs = ep_sb.tile([64, 512], F32, tag="bcs")
                nc.vector.tensor_copy(bcs[:], bcp[0:64, 0:512])
                nc.vector.tensor_mul(
                    ctxT[ds(64 * hh, 64), ts(qi, 512)],
                    cp[0:64, :], bcs[:],
                )

            # rolling queues: ctx matmuls trail the score/exp stream by 2
            # pairs (covers exp + mask-select latency) and may cross stage
            # boundaries; each head's softmax epilogue trails by one stage.
            pending_epi = None
            cp_q = []  # (stage_no, emit_fn)

            def pump_to(keep):
                while len(cp_q) > keep:
                    cp_q.pop(0)[1]()

            def pump_stage(s):
                while cp_q and cp_q[0][0] <= s:
                    cp_q.pop(0)[1]()

            stage = 0
            for qi in range(NQ):
                nk = 4 * qi + 4  # lower-triangular k-chunks
                for hh in range(HL):
                    cp = cp_ps.tile([65, 512], F32, tag="cp")
                    for kp_ in range(2 * qi):  # below-diagonal ki pairs
                        sp = sp_ps.tile([128, 1024], F32, tag="sp")
                        for h in range(2):
                            ki = 2 * kp_ + h
                            half = sp[:, ds(h * 512, 512)]
                            nc.tensor.matmul(
                                half,
                                kt[ds(64 * hh, 64), ts(ki, 128)].opt(),
                                qt[ds(64 * hh, 64), ts(qi, 512)].opt(),
                                start=True, stop=True,
                            )
                        et = et_sb.tile([128, 1024], BF16, tag="et")
                        nc.scalar.activation(et[:], sp[:], EXP, scale=0.125)

                        def emit_cp(kp_=kp_, et=et, cp=cp, hh=hh):
                            for h in range(2):
                                ki = 2 * kp_ + h
                                nc.tensor.matmul(
                                    cp[:], vaug[:, ki, ds(65 * hh, 65)],
                                    et[:, ds(h * 512, 512)],
                                    start=(ki == 0), stop=False,
                                    skip_group_check=True,
                                )
                        cp_q.append((stage, emit_cp))
                        pump_to(2)
                    # diagonal 512x512 block at 128-query granularity:
                    # diag chunk kk attends query sub-chunks t >= kk only
                    for kk in range(4):
                        ki = 4 * qi + kk
                        w = (4 - kk) * 128
                        sp = sp_ps.tile([128, 512], F32, tag="sp")
                        for t in range(kk, 4):
                            nc.tensor.matmul(
                                sp[:, ds((t - kk) * 128, 128)],
                                kt[ds(64 * hh, 64), ts(ki, 128)].opt(),
                                qt[ds(64 * hh, 64),
                                   ds(qi * 512 + t * 128, 128)].opt(),
                                start=True, stop=True,
                            )
                        et = et_sb.tile([128, 512], BF16, tag="et")
                        nc.scalar.activation(et[:, 0:w], sp[:, 0:w],
                                             EXP, scale=0.125)
                        # true-diagonal 128-block: zero et[k, y] where y < k
                        nc.gpsimd.affine_select(
                            out=et[:, 0:128], in_=et[:, 0:128],
                            compare_op=mybir.AluOpType.is_ge,
                            fill=0.0, base=0, pattern=[[1, 128]],
                            channel_multiplier=-1,
                        )

                        def emit_cp_diag(kk=kk, ki=ki, w=w, et=et, cp=cp,
                                         hh=hh, qi=qi):
                            nc.tensor.matmul(
                                cp[:, ds(kk * 128, w)],
                                vaug[:, ki, ds(65 * hh, 65)],
                                et[:, 0:w],
                                start=(qi == 0 and kk == 0), stop=(kk == 3),
                                skip_group_check=True,
                            )
                        cp_q.append((stage, emit_cp_diag))
                        pump_to(2)
                    if pending_epi is not None:
                        pump_stage(stage - 1)
                        epilogue(*pending_epi)
                        pending_epi = None
                    if half_cb is not None and qi == NQ - 1 and hh == 0:
                        # eager epilogue so the first partition-half of the
                        # final A2A overlaps the last head's compute
                        pump_stage(stage)
                        epilogue(cp, hh, qi)
                        half_cb()
                    else:
                        pending_epi = (cp, hh, qi)
                    stage += 1
                if qi in (1, 3):
                    pump_stage(stage - 1)
                    epilogue(*pending_epi)
                    pending_epi = None
                    (mid_cb if qi == 1 else end_cb)()

        # warmup collective: absorbs the one-time CC init cost during QKV(0)
        wu_in = nc.dram_tensor("wu_in", [NC, 128, 4], BF16)
        wu_out = nc.dram_tensor("wu_out", [NC, 128, 4], BF16)
        wu = const.tile([128, NC * 4], BF16)
        nc.vector.memset(wu[:], 0.0)
        nc.sync.dma_start(wu_in[:], wu[:].rearrange("p (j n) -> j p n", j=NC))
        nc.gpsimd.collective_compute(
            "AllToAll", mybir.AluOpType.bypass,
            replica_groups=[list(range(NC))],
            ins=[wu_in[:]], outs=[wu_out[:]],
        )

        pending = []  # triggered but not yet consumed A2A ids

        def consume_oldest(keep):
            while len(pending) > keep:
                consume_a2a(pending.pop(0))

        for b in range(B):
            ctxT = ctx_sb.tile([128, S], BF16, tag="ctx")
            qt = qk_sb.tile([128, S], BF16, tag="qt")
            kt = qk_sb.tile([128, S], BF16, tag="kt")
            vaug = vpool.tile([128, NKC, 130], BF16)
            qkv_batch(b, qt, kt, vaug, first=(b == 0))

            def mid_cb(b=b, ctxT=ctxT):
                trigger_a2a(2 * b, ctxT, 0)
                if b == 0:
                    nc.sync.dma_start(
                        wo_sb[:], wo.rearrange("(j p) n -> p j n", p=128))
                pending.append(2 * b)
                consume_oldest(2)

            def end_cb(b=b, ctxT=ctxT):
                if b == B - 1:
                    trigger_a2a_half(1, ctxT, 1)
                else:
                    trigger_a2a(2 * b + 1, ctxT, 1)
                pending.append(2 * b + 1)
                consume_oldest(2)

            def half_cb(ctxT=ctxT):
                trigger_a2a_half(0, ctxT, 1)

            attn_batch(b, qt, kt, vaug, ctxT, mid_cb, end_cb,
                       half_cb if b == B - 1 else None)
        consume_oldest(0)

    return nc



# ======== host-side wrapper ========
_CACHE = {}


def _get_program():
    if "nc" not in _CACHE:
        install()
        _CACHE["nc"] = build()
    return _CACHE["nc"]


def _run(inputs, trace=False):
    import ml_dtypes

    from concourse.bass_utils import run_bass_kernel_spmd

    bf16 = ml_dtypes.bfloat16
    x = np.asarray(inputs["x"], dtype=np.float32)
    WQ = np.asarray(inputs["WQ"], dtype=np.float32)
    WK = np.asarray(inputs["WK"], dtype=np.float32)
    WV = np.asarray(inputs["WV"], dtype=np.float32)
    WO = np.asarray(inputs["WO"], dtype=np.float32)

    xTh = np.ascontiguousarray(x.reshape(BS, D).T.astype(bf16))
    woT = np.ascontiguousarray(WO.T.astype(bf16))
    in_maps = []
    for c in range(NC):
        sl = slice(c * 128, (c + 1) * 128)
        in_maps.append({
            "xT": xTh,
            "wq": np.ascontiguousarray(WQ[sl, :].T.astype(bf16)),
            "wk": np.ascontiguousarray(WK[sl, :].T.astype(bf16)),
            "wv": np.ascontiguousarray(WV[sl, :].T.astype(bf16)),
            "wo": woT,
        })

    nc_prog = _get_program()
    res = run_bass_kernel_spmd(nc_prog, in_maps, list(range(NC)), trace=trace)

    actual = np.zeros((BS, D), dtype=np.float32)
    for c in range(NC):
        oc = res.results[c]["out"]
        for b in range(B):
            for h in range(2):
                t = 2 * b + h
                r0 = b * S + h * 1024 + c * STR
                actual[r0:r0 + STR] = oc[(t * STR):(t + 1) * STR]
    return actual.reshape(x.shape), res


def kernel(**inputs):
    out, _ = _run(inputs, trace=False)
    return out

